# revision 20
# baseline (speedup 1.0000x reference)
"""3-layer GCN (PyG gcn_norm semantics) on 8 Trainium2 NeuronCores.

Sharding: nodes are range-partitioned across the 8 cores (graph parallel);
weights are replicated.  The gather table stores z_i = zscale*dis_i*y_i
(dis = deg^-1/2, y = a @ W) in fp8(e4m3) so the one-hot scatter matrix is
EXACTLY {0,1} in fp8 (self-loops are plain edges).  dis_d on the
destination side is a free-axis DVE multiply in the epilogue; dis_i on
the source side is a per-partition scalar fused into the psum->fp8 cast.

Each layer table is split into TWO Shared-DRAM halves (A = first 6272
node rows of every core, B = the rest), each written by exactly ONE fast
Shared AllGather.  Edges are bucketed on the host by (dst chunk, src
half) and each layer runs two passes: pass A processes all src-half-A
edge blocks (partial segment-sums staged to SBUF fp16), pass B the rest,
adding the staged partials in the epilogue.  AG(A of next table) issues
halfway through pass B, AG(B) at layer end - both hide behind compute,
so no layer-boundary collective stall remains.

Message matmuls run in fp8 DoubleRow mode (256 edges per PE instruction;
LDWEIGHTS overlaps MATMUL).  Chunk epilogues are batched in groups of 4
chunks (one [128,256] PSUM tile, whole-group DVE/ACT ops, two 128-wide
fp16 weight matmuls, one 256-row shard DMA).  The final layer writes
logits into one SBUF tile; log_softmax runs batched over all 98 node
pairs with 3D access patterns and ONE 0.5MB output DMA (host permutes).

Host-side work is limited to sharding/index preprocessing and layout
permutes of inputs/outputs.
"""

import os

import numpy as np

import concourse.bass as bass
import concourse.bacc as bacc
import concourse.mybir as mybir
import concourse.tile as tile
from concourse.bass import ts
from concourse.bass_utils import run_bass_kernel_spmd

F32 = mybir.dt.float32
F16 = mybir.dt.float16
I32 = mybir.dt.int32
P = 128  # partition dim == feature dim

LAST_RESULT = None


class Cfg:
    def __init__(self, n_cores, n_nodes, n_class, gather_k,
                 table_dtype=mybir.dt.float8e4, cw=64, zscale=8.0,
                 double_row=True, grp=4, l0g=7):
        assert n_nodes % n_cores == 0
        self.n_cores = n_cores
        self.n_nodes = n_nodes
        self.n_class = n_class
        self.gather_k = gather_k
        self.cw = cw                         # scatter window (chunk) width
        self.S = n_nodes // n_cores          # rows per core
        self.CH = -(-self.S // cw)           # cw-row chunks per core
        self.S_pad = self.CH * cw
        assert self.S_pad % P == 0
        self.T_half = n_cores * (self.S_pad // 2)
        self.prow = self.S_pad // 2          # nodes per table half per core
        assert self.prow % P == 0
        self.table_dtype = table_dtype
        self.zscale = zscale
        self.double_row = double_row and table_dtype == mybir.dt.float8e4
        self.grp = grp                       # chunks per epilogue group
        assert self.CH % grp == 0
        self.ngrp = self.CH // grp
        assert (grp * cw) % P == 0
        self.l0g = l0g                       # layer-0 tiles per staging DMA
        assert (self.S_pad // P) % l0g == 0
        assert (self.prow // P) % l0g == 0

    @property
    def np_table(self):
        return np.dtype(mybir.dt.np(self.table_dtype))


FULL = Cfg(n_cores=8, n_nodes=100000, n_class=10, gather_k=32)


def _preprocess(cfg, edge_index):
    """Bucket edges by (dst chunk, src half), build block-padded arrays.

    Returns dict with NBa/NBb, per-pass matmul op lists
    (chunk, block, nk, first, last) and per-core eidx/eoh arrays.
    """
    S, CH, K = cfg.S, cfg.CH, cfg.gather_k
    n = cfg.n_nodes
    src = np.concatenate([edge_index[0], np.arange(n, dtype=np.int64)])
    dst = np.concatenate([edge_index[1], np.arange(n, dtype=np.int64)])

    score = src // S
    r = src - score * S
    half = r // cfg.prow                                   # src table half
    srcrow = (half * cfg.T_half + score * cfg.prow
              + (r - half * cfg.prow)).astype(np.int32)

    core = dst // S
    dloc = dst - core * S
    chunk = dloc // cfg.cw
    w = dloc % cfg.cw

    kidx = ((core * CH + chunk) * 2 + half).astype(np.int64)
    counts = np.bincount(kidx, minlength=cfg.n_cores * CH * 2)
    counts = counts.reshape(cfg.n_cores, CH, 2)
    Bcp = np.maximum(1, -(-counts.max(axis=0) // P)).astype(np.int64)  # [CH,2]
    NBa = int(Bcp[:, 0].sum())
    Bcp[-1, 0] += (-NBa) % K
    NBa = int(Bcp[:, 0].sum())
    NBb = int(Bcp[:, 1].sum())
    Bcp[-1, 1] += (-NBb) % K
    NBb = int(Bcp[:, 1].sum())
    NB = NBa + NBb

    offs = np.zeros((CH, 2), np.int64)
    offs[1:, 0] = np.cumsum(Bcp[:, 0])[:-1]
    offs[1:, 1] = np.cumsum(Bcp[:, 1])[:-1]
    offs[:, 1] += NBa

    order = np.argsort(kidx, kind="stable")
    skey = kidx[order]
    gstart = np.searchsorted(skey, np.arange(cfg.n_cores * CH * 2))
    ranks = np.arange(len(order)) - gstart[skey]
    ch_s = (skey // 2) % CH
    p_s = skey % 2
    core_s = skey // (2 * CH)
    slots = offs[ch_s, p_s] * P + ranks

    L = NB * P
    td = cfg.np_table
    idx_a = np.zeros((cfg.n_cores, L), np.int32)
    w_a = np.full((cfg.n_cores, L), -1, np.int64)
    idx_a[core_s, slots] = srcrow[order]
    w_a[core_s, slots] = w[order]

    def make_ops(nb_arr, base):
        ops = []
        b = base
        for c in range(CH):
            s = b
            e = b + int(nb_arr[c])
            i = s
            while i < e:
                nk = 2 if (cfg.double_row and i + 1 < e
                           and i // K == (i + 1) // K) else 1
                ops.append((c, i, nk, i == s, i + nk == e))
                i += nk
            b = e
        assert b == base + int(nb_arr.sum())
        return ops

    opsA = make_ops(Bcp[:, 0], 0)
    opsB = make_ops(Bcp[:, 1], NBa)

    per_core = []
    ar = np.arange(L)
    for c in range(cfg.n_cores):
        oh = np.zeros((L, cfg.cw), td)
        m = w_a[c] >= 0
        oh[ar[m], w_a[c][m]] = td.type(1.0)
        oh = np.ascontiguousarray(
            oh.reshape(NB, P, cfg.cw).transpose(1, 0, 2).reshape(P, NB * cfg.cw)
        )
        per_core.append({
            "eidx": np.ascontiguousarray(idx_a[c].reshape(NB, P).T),
            "eoh": oh,
        })
    return {"NBa": NBa, "NBb": NBb, "opsA": opsA, "opsB": opsB,
            "per_core": per_core}


def _build_program(cfg, pp):
    nc = bacc.Bacc(
        "TRN2", target_bir_lowering=False, debug=False, num_devices=cfg.n_cores
    )
    CH, K, NC = cfg.CH, cfg.gather_k, cfg.n_class
    CW = cfg.cw
    TD = cfg.table_dtype
    GRP = cfg.grp
    GN = GRP * CW                   # nodes per epilogue group (256)
    NPAIR = cfg.S_pad // P          # 128-node pairs per core (98)
    NBa, NBb = pp["NBa"], pp["NBb"]
    NB = NBa + NBb
    rg = [list(range(cfg.n_cores))]
    DRM = mybir.MatmulPerfMode.DoubleRow

    xT_in = nc.dram_tensor("xT", [P, cfg.S_pad], F16, kind="ExternalInput")
    eidx_in = nc.dram_tensor("eidx", [P, NB], I32, kind="ExternalInput")
    eoh_in = nc.dram_tensor("eoh", [P, NB * CW], TD, kind="ExternalInput")
    W_in = [
        nc.dram_tensor(f"W{i + 1}", [P, P], F16, kind="ExternalInput")
        for i in range(3)
    ]
    Wl_in = nc.dram_tensor("Wl", [P, NC], F16, kind="ExternalInput")
    b_in = [
        nc.dram_tensor(f"b{i + 1}", [P, 1], F32, kind="ExternalInput")
        for i in range(3)
    ]
    blT_in = nc.dram_tensor("blT", [P, NC], F32, kind="ExternalInput")
    disbc_in = nc.dram_tensor("disbc", [P, cfg.S_pad], F16, kind="ExternalInput")
    disz0_in = nc.dram_tensor("disz0", [P, NPAIR], F32, kind="ExternalInput")
    out_t = nc.dram_tensor("logits", [P, NPAIR * NC], F32, kind="ExternalOutput")
    DEBUG = bool(os.environ.get("GCN_DEBUG"))
    if DEBUG:
        dbg_shard0 = nc.dram_tensor("dbg_shard0", [cfg.S_pad, P], F32,
                                    kind="ExternalOutput")
        dbg_part = nc.dram_tensor("dbg_part", [P, cfg.S_pad], F32,
                                  kind="ExternalOutput")
        dbg_olog = nc.dram_tensor("dbg_olog", [P, NPAIR * NC], F32,
                                  kind="ExternalOutput")
        dbg_gt = nc.dram_tensor("dbg_gt", [P, cfg.gather_k * P], F32,
                                kind="ExternalOutput")

    with tile.TileContext(nc) as tc:
        with (
            tc.tile_pool(name="const", bufs=1) as constp,
            tc.tile_pool(name="persist", bufs=1) as persist,
            tc.tile_pool(name="gather", bufs=6) as gatherp,
            tc.tile_pool(name="ohp", bufs=6) as ohp,
            tc.tile_pool(name="epi", bufs=4) as epip,
            tc.tile_pool(name="mpsum", bufs=3, space="PSUM") as mpsump,
            tc.tile_pool(name="opsum", bufs=4, space="PSUM") as opsump,
            tc.tile_pool(name="dram", bufs=1, space="DRAM") as dramp,
        ):
            # layer-0-critical loads first on the sync DMA queue
            xT_t = persist.tile([P, cfg.S_pad], F16)
            nc.sync.dma_start(xT_t[:], xT_in[:])
            W_t = []
            for i in range(3):
                wt = constp.tile([P, P], F16, name=f"w{i}")
                nc.sync.dma_start(wt[:], W_in[i][:])
                W_t.append(wt)
            disz0_t = constp.tile([P, NPAIR], F32)
            nc.sync.dma_start(disz0_t[:], disz0_in[:])
            # the rest on the scalar DMA queue
            Wl_t = constp.tile([P, NC], F16)
            nc.scalar.dma_start(Wl_t[:], Wl_in[:])
            b_t = []
            for i in range(3):
                bt = constp.tile([P, 1], F32, name=f"b{i}")
                nc.scalar.dma_start(bt[:], b_in[i][:])
                b_t.append(bt)
            blT_t = constp.tile([P, NC], F32)
            nc.scalar.dma_start(blT_t[:], blT_in[:])
            disbc_t = persist.tile([P, cfg.S_pad], F16)
            nc.scalar.dma_start(disbc_t[:], disbc_in[:])
            idx_t = persist.tile([P, NB], I32)
            nc.scalar.dma_start(idx_t[:], eidx_in[:])

            part_t = persist.tile([P, cfg.S_pad], F16)   # pass-A partials
            olog_t = persist.tile([P, NPAIR * NC], F32)  # final logits

            tbl_shard = [
                dramp.tile([cfg.S_pad, P], TD, name=f"shard{i}") for i in range(3)
            ]
            # single table tensor per layer; the two AllGathers write
            # disjoint halves ([0, T_half) and [T_half, 2*T_half))
            tbl_full = [
                dramp.tile([2 * cfg.T_half, P], TD, name=f"full{i}")
                for i in range(3)
            ]

            def ag(l, h):
                nc.gpsimd.collective_compute(
                    "AllGather", mybir.AluOpType.bypass, replica_groups=rg,
                    ins=[tbl_shard[l][h * cfg.prow:(h + 1) * cfg.prow, :].opt()],
                    outs=[
                        tbl_full[l][
                            h * cfg.T_half:(h + 1) * cfg.T_half, :
                        ].opt()
                    ],
                )

            def stage_dma(l, row0, nrow, st):
                """st [P, (nrow/128)*P] fp8 -> shard rows row0..+nrow."""
                for q in range(nrow // P):
                    nc.sync.dma_start(
                        tbl_shard[l][row0 + q * P:row0 + (q + 1) * P, :],
                        st[:, ts(q, P)],
                    )

            # layer 0: z0 = zscale*dis*(x @ W1), staged per l0g tiles
            L0G = cfg.l0g
            st0 = None
            for t in range(cfg.S_pad // P):
                hp = opsump.tile([P, P], F32, name="hp", tag="o")
                nc.tensor.matmul(
                    hp[:], lhsT=xT_t[:, ts(t, P)], rhs=W_t[0][:],
                    start=True, stop=True,
                )
                if t % L0G == 0:
                    st0 = epip.tile([P, L0G * P], TD, name="st0")
                nc.vector.tensor_scalar(
                    st0[:, (t % L0G) * P:(t % L0G + 1) * P], hp[:],
                    disz0_t[:, t:t + 1], None, mybir.AluOpType.mult
                )
                if (t + 1) % L0G == 0:
                    stage_dma(0, (t // L0G) * L0G * P, L0G * P, st0)
                    if (t + 1) * P == cfg.prow:
                        ag(0, 0)
                    elif (t + 1) * P == cfg.S_pad:
                        ag(0, 1)

            # message-passing layers, two passes (src half A then B)
            for l in range(3):
                pend = []

                def flush(l=None):
                    for grp, aT in pend:
                        if l < 2:
                            st = epip.tile([P, GN], TD, name="hbst")
                            for q in range(GN // P):
                                pair = grp * (GN // P) + q
                                hp2 = opsump.tile(
                                    [P, P], F32, name="hp2", tag="o")
                                nc.tensor.matmul(
                                    hp2[:], lhsT=aT[:, ts(q, P)],
                                    rhs=W_t[l + 1][:], start=True, stop=True,
                                )
                                nc.vector.tensor_scalar(
                                    st[:, ts(q, P)], hp2[:],
                                    disz0_t[:, pair:pair + 1], None,
                                    mybir.AluOpType.mult,
                                )
                            stage_dma(l + 1, grp * GN, GN, st)
                            if grp == (cfg.prow - 1) // GN:
                                ag(l + 1, 0)
                            elif grp == cfg.ngrp - 1:
                                ag(l + 1, 1)
                        else:
                            for q in range(GN // P):
                                pair = grp * (GN // P) + q
                                lp = opsump.tile(
                                    [P, NC], F32, name="lp", tag="o",
                                    padded_shape=[P, P],
                                )
                                nc.tensor.matmul(
                                    lp[:], lhsT=aT[:, ts(q, P)], rhs=Wl_t[:],
                                    start=True, stop=True,
                                )
                                nc.vector.tensor_tensor(
                                    olog_t[:, pair * NC:(pair + 1) * NC],
                                    lp[:], blT_t[:], mybir.AluOpType.add,
                                )
                    pend.clear()

                for half in range(2):
                    ops = pp["opsA"] if half == 0 else pp["opsB"]
                    base = 0 if half == 0 else NBa
                    npass = (NBa if half == 0 else NBb)
                    full_t = tbl_full[l]
                    gt = oh = cur_psum = None
                    cur_g = -1
                    for cid, bst, nk, first, last in ops:
                        g = bst // K
                        if g != cur_g:
                            cur_g = g
                            gt = gatherp.tile([P, K * P], TD, name="gt")
                            nc.gpsimd.indirect_dma_start(
                                out=gt[:], out_offset=None,
                                in_=full_t[:],
                                in_offset=bass.IndirectOffsetOnAxis(
                                    ap=idx_t[:, g * K:(g + 1) * K], axis=0
                                ),
                            )
                            oh = ohp.tile([P, K * CW], TD, name="oh")
                            nc.scalar.dma_start(
                                oh[:], eoh_in[:, g * K * CW:(g + 1) * K * CW]
                            )
                        if DEBUG and l == 0 and half == 0 and g == 0 and bst == 0:
                            gtc = epip.tile([P, K * P], F32, name="gtc", bufs=1)
                            nc.vector.tensor_copy(gtc[:], gt[:])
                            nc.sync.dma_start(dbg_gt[:], gtc[:])
                        j = bst - g * K
                        ci = cid % GRP
                        if first and ci == 0:
                            cur_psum = mpsump.tile([P, GN], F32, name="msg")
                        out_ap = cur_psum[:, ci * CW:(ci + 1) * CW]
                        if nk == 2:
                            nc.tensor.matmul(
                                out_ap,
                                lhsT=gt[:, j * P:(j + 2) * P].rearrange(
                                    "p (k m) -> p k m", k=2),
                                rhs=oh[:, j * CW:(j + 2) * CW].rearrange(
                                    "p (k n) -> p k n", k=2),
                                start=first, stop=last, perf_mode=DRM,
                            )
                        else:
                            nc.tensor.matmul(
                                out_ap, lhsT=gt[:, ts(j, P)],
                                rhs=oh[:, ts(j, CW)], start=first, stop=last,
                            )
                        if not (last and ci == GRP - 1):
                            continue
                        grp = cid // GRP
                        gsl = slice(grp * GN, (grp + 1) * GN)
                        if half == 0:
                            # stage pass-A partial to SBUF fp16
                            nc.scalar.activation(
                                part_t[:, gsl], cur_psum[:],
                                mybir.ActivationFunctionType.Copy,
                            )
                            continue
                        # pass-B epilogue for the 4-chunk group
                        s_t = epip.tile([P, GN], F32, name="s_t")
                        nc.vector.tensor_tensor(
                            s_t[:], cur_psum[:], part_t[:, gsl],
                            mybir.AluOpType.add,
                        )
                        t2 = epip.tile([P, GN], F32, name="t2")
                        nc.vector.tensor_tensor(
                            t2[:], s_t[:], disbc_t[:, gsl],
                            mybir.AluOpType.mult,
                        )
                        aT = epip.tile([P, GN], F16, name="aT", bufs=6)
                        if l < 2:
                            nc.scalar.activation(
                                aT[:], t2[:],
                                mybir.ActivationFunctionType.Relu,
                                bias=b_t[l][:, :1],
                            )
                        else:
                            nc.vector.tensor_scalar(
                                aT[:], t2[:], b_t[2][:, :1], None,
                                mybir.AluOpType.add,
                            )
                        pend.append((grp, aT))
                        if len(pend) >= 2:
                            flush(l)
                    assert cur_g == npass // K - 1 + (base // K), (
                        cur_g, npass, base)
                    flush(l)
                    if DEBUG and l == 0 and half == 0:
                        pc = epip.tile([P, 2048], F32, name="pc", bufs=1)
                        nc.vector.tensor_copy(pc[:], part_t[:, :2048])
                        nc.sync.dma_start(dbg_part[:, :2048], pc[:])

            if DEBUG:
                oc = persist.tile([P, NPAIR * NC], F32, name="oc")
                nc.vector.tensor_copy(oc[:], olog_t[:])
                nc.sync.dma_start(dbg_olog[:], oc[:])
                ld8 = persist.tile([P, 16 * P], TD, name="ld8")
                cc8 = persist.tile([P, 16 * P], F32, name="cc8")
                for q in range(16):
                    nc.sync.dma_start(
                        ld8[:, ts(q, P)], tbl_shard[0][ts(q, P), :]
                    )
                nc.vector.tensor_copy(cc8[:], ld8[:])
                for q in range(16):
                    nc.sync.dma_start(
                        dbg_shard0[ts(q, P), :], cc8[:, ts(q, P)]
                    )

            # batched log_softmax over all 98 node pairs
            v3 = olog_t[:, :].rearrange("p (c k) -> p c k", c=NPAIR)
            mx_t = persist.tile([P, NPAIR], F32)
            nc.vector.reduce_max(mx_t[:], v3, axis=mybir.AxisListType.X)
            osub_t = persist.tile([P, NPAIR * NC], F32)
            nc.vector.tensor_tensor(
                osub_t[:, :].rearrange("p (c k) -> p c k", c=NPAIR), v3,
                mx_t[:, :, None].broadcast_to((P, NPAIR, NC)),
                mybir.AluOpType.subtract,
            )
            ex_t = persist.tile([P, NPAIR * NC], F32)
            nc.scalar.activation(
                ex_t[:], osub_t[:], mybir.ActivationFunctionType.Exp
            )
            sums_t = persist.tile([P, NPAIR], F32)
            nc.vector.reduce_sum(
                sums_t[:], ex_t[:, :].rearrange("p (c k) -> p c k", c=NPAIR),
                axis=mybir.AxisListType.X,
            )
            ln_t = persist.tile([P, NPAIR], F32)
            nc.scalar.activation(
                ln_t[:], sums_t[:], mybir.ActivationFunctionType.Ln
            )
            fin_t = persist.tile([P, NPAIR * NC], F32)
            nc.vector.tensor_tensor(
                fin_t[:, :].rearrange("p (c k) -> p c k", c=NPAIR),
                osub_t[:, :].rearrange("p (c k) -> p c k", c=NPAIR),
                ln_t[:, :, None].broadcast_to((P, NPAIR, NC)),
                mybir.AluOpType.subtract,
            )
            nc.sync.dma_start(out_t[:], fin_t[:])
    nc.compile()
    return nc


def _make_in_maps(cfg, pp, x, edge_index, W1, b1, W2, b2, W3, b3, Wl, bl):
    n = cfg.n_nodes
    dst = np.concatenate([edge_index[1], np.arange(n, dtype=np.int64)])
    deg = np.bincount(dst, minlength=n).astype(np.float64)
    dis = (1.0 / np.sqrt(deg)).astype(np.float32)

    shared = {
        "W1": np.asarray(W1, np.float16),
        "W2": np.asarray(W2, np.float16),
        "W3": np.asarray(W3, np.float16),
        "Wl": np.asarray(Wl, np.float16),
        "b1": np.asarray(b1, np.float32).reshape(P, 1),
        "b2": np.asarray(b2, np.float32).reshape(P, 1),
        "b3": np.asarray(b3, np.float32).reshape(P, 1),
        "blT": np.broadcast_to(
            np.asarray(bl, np.float32)[None, :], (P, cfg.n_class)
        ).copy(),
    }
    in_maps = []
    for c in range(cfg.n_cores):
        dl = np.zeros(cfg.S_pad, np.float32)
        dl[: cfg.S] = dis[c * cfg.S:(c + 1) * cfg.S]
        xs = np.zeros((P, cfg.S_pad), np.float16)
        xs[:, : cfg.S] = np.asarray(x[c * cfg.S:(c + 1) * cfg.S], np.float16).T
        disbc = np.broadcast_to(
            (dl / cfg.zscale).astype(np.float16)[None, :], (P, cfg.S_pad)
        )
        disz = (dl * cfg.zscale).astype(np.float32)
        in_maps.append({
            "xT": np.ascontiguousarray(xs),
            "eidx": pp["per_core"][c]["eidx"],
            "eoh": pp["per_core"][c]["eoh"],
            "disbc": np.ascontiguousarray(disbc),
            "disz0": np.ascontiguousarray(
                disz.reshape(cfg.S_pad // P, P).T
            ),
            **shared,
        })
    return in_maps


def kernel(x, edge_index, W1, b1, W2, b2, W3, b3, Wl, bl):
    cfg = FULL
    x = np.asarray(x)
    edge_index = np.asarray(edge_index)
    pp = _preprocess(cfg, edge_index)
    nc = _build_program(cfg, pp)
    in_maps = _make_in_maps(
        cfg, pp, x, edge_index, W1, b1, W2, b2, W3, b3, Wl, bl
    )
    res = run_bass_kernel_spmd(
        nc, in_maps, list(range(cfg.n_cores)),
        trace=bool(os.environ.get("GCN_TRACE")),
    )
    global LAST_RESULT
    LAST_RESULT = res
    out = np.empty((cfg.n_nodes, cfg.n_class), np.float32)
    NPAIR = cfg.S_pad // P
    for c in range(cfg.n_cores):
        r = np.asarray(res.results[c]["logits"])  # [P, NPAIR*NC]
        r = r.reshape(P, NPAIR, cfg.n_class).transpose(1, 0, 2)
        out[c * cfg.S:(c + 1) * cfg.S] = r.reshape(
            cfg.S_pad, cfg.n_class)[: cfg.S]
    return out
